# revision 41
# baseline (speedup 1.0000x reference)
"""MoE routing kernel for 8 TRN2 NeuronCores.

reference:
    h = relu(x @ W1 + b1)            # [B, 512]
    e = c[num]                       # [B] expert ids
    out = sigmoid(h @ We[e] + be[e]) # [B, 128]

Strategy: data-parallel over B with host-side expert sort.  Rows are
stable-sorted by expert id, each expert's row count is padded to a
multiple of 8, and the sorted rows are dealt round-robin to the 8 cores.
Because every expert boundary lands on a multiple of 8 globally, all 8
cores see the *same* local expert-boundary structure, so one SPMD graph
(with per-512-row-group expert segments baked in as compile-time
constants) is valid for every core.  x is pre-transposed on the host so
the device contracts over the partition axis with zero on-device
transposes; the device returns out^T in bf16 which the host transposes
back to f32.

Both GEMMs run in fp8 e4m3 with DoubleRow perf mode (2 contraction rows
per cycle): x/W1 quantized host-side, h quantized on the fly by the
PSUM-drain engines, We quantized host-side with a x16 scale (folded
back out via the sigmoid's scale operand) to dodge the e4m3 subnormal
floor.  The ReLU+bias drain is split across VectorE (hc 0-1) and
ScalarE (hc 2-3) so neither engine gates the fp8-rate PE; sigmoid+bias
runs on ScalarE.  ~48 dummy matmuls on zeroed SBUF warm the PE's HAM
clock gate during the initial DMA wait, and the first loads ride three
parallel HWDGE queues (sync/gpsimd/scalar) so queue spin-up and the
W1/x0 transfers overlap.
"""

import ml_dtypes
import numpy as np

import concourse.bass as bass
import concourse.mybir as mybir
from concourse import tile
from concourse import bass_utils

B, D_IN, D_H, D_OUT, N_EXP = 65536, 512, 512, 128, 16
NCORES = 8
GROUP = 512  # rows per matmul group (one PSUM bank of fp32)
KC = D_IN // 128   # 4 contraction chunks for the trunk
HC = D_H // 128    # 4 contraction chunks for the expert matmul

F8 = ml_dtypes.float8_e4m3  # TRN fp8_exp4 (max +-240), bit-compatible

# "fp8": expert GEMM fully fp8 DoubleRow (fastest, rel err ~1.97e-2)
# "mixed": hc 0-1 fp8 DoubleRow + hc 2-3 bf16 (rel err ~1.7e-2)
# "bf16": expert GEMM fully bf16 (rel err ~1.4e-2)
EXPERT_MODE = "fp8"
WE_SCALE = 16.0  # host-side We scale; 1/WE_SCALE folded into sigmoid
# Dummy matmuls bridging the PE from preamble-end (~8.4us) to the first
# x/W1 data (~12us): keeps the HAM activity monitor busy so the real
# matmuls start at 2.4 GHz instead of cold 1.2 GHz, at zero cost while
# the DMA queues ramp.  ~213ns each (N=256 cold).
N_WARM = 23
PREFETCH = 4     # x tile for bundle k+PREFETCH issued during bundle k

# test.py introspection: the last BassKernelResults (for exec_time_ns)
LAST_RESULTS = None

# If profiling is enabled via BASS_TRACE, keep artifacts local (the default
# upload path needs a remote bucket this environment may not have).
bass_utils.upload_artifacts = lambda tmpdir: tmpdir


def _split_waits(nc, limit=1):
    """Walrus's CoreV3 CTRL codegen rejects instructions carrying more
    than one sem wait; spread extras onto preceding same-engine NoOps."""
    for f in nc.m.functions:
        for bb in f.blocks:
            insts = list(bb.instructions)
            out = []
            changed = False
            for ins in insts:
                si = ins.sync_info
                waits = list(si.on_wait) if si and si.on_wait else []
                if len(waits) > limit:
                    extra, keep = waits[:-limit], waits[-limit:]
                    for i in range(0, len(extra), limit):
                        out.append(
                            mybir.InstNoOp(
                                name=f"{ins.name}-ws{i}",
                                engine=ins.engine,
                                ins=[],
                                outs=[],
                                sync_info=mybir.SyncInfo(
                                    on_wait=extra[i : i + limit], on_update=[]
                                ),
                            )
                        )
                    ins.sync_info = mybir.SyncInfo(
                        on_wait=keep,
                        on_update=list(si.on_update) if si.on_update else [],
                    )
                    changed = True
                out.append(ins)
            if changed:
                bb.instructions[:] = out


def _strip_exit_barriers(nc):
    """Drop Tile's exit-time double all-engine barrier + DMA-queue reset
    + semaphore clear (~8us of measured postamble, inside the profiled
    exec window).  The NEFF executes once per kernel() call, so
    inter-execution semaphore hygiene is dead weight.  The final sync
    drain — which waits out the global tile clock, including the
    out-DMA completion sems — is kept, so outputs are still complete
    when the program ends."""
    for f in nc.m.functions:
        for bb in f.blocks:
            if not bb.name.endswith("_end"):
                continue
            insts = list(bb.instructions)
            cut = None
            for idx, ins in enumerate(insts):
                if (
                    isinstance(ins, mybir.InstDrain)
                    and ins.engine == mybir.EngineType.SP
                ):
                    cut = idx
                    break
            if cut is not None:
                bb.instructions[:] = insts[: cut + 1]


def _build_graph(R, bundles, expert_order):
    """Build the per-core Bass graph.

    R: local rows per core (multiple of 8).
    bundles: list of bundles; a bundle is a list of (gstart, glen, segs)
    with segs = [(expert, s0, slen)] local to the group.
    expert_order: used experts in first-use order; We is packed in this
    order host-side so the early slots can land first.
    """
    f32 = mybir.dt.float32
    bf16 = mybir.dt.bfloat16
    f8 = mybir.dt.float8e4
    DR = mybir.MatmulPerfMode.DoubleRow

    n_used = len(expert_order)
    slot_of = {e: s for s, e in enumerate(expert_order)}

    nc = bass.Bass()
    # xT carries W1 as a same-layout prefix: W1 = xT[:, :, :D_H].
    xT = nc.declare_dram_parameter("xT", [128, KC, D_H + R], f8, isOutput=False)
    if EXPERT_MODE == "fp8":
        WerA = nc.declare_dram_parameter(
            "WerA", [128, n_used * HC, D_OUT], f8, isOutput=False
        )
        WerB = None
    elif EXPERT_MODE == "mixed":
        WerA = nc.declare_dram_parameter(
            "WerA", [128, n_used * 2, D_OUT], f8, isOutput=False
        )
        WerB = nc.declare_dram_parameter(
            "WerB", [128, n_used * 2, D_OUT], bf16, isOutput=False
        )
    else:
        WerA = None
        WerB = nc.declare_dram_parameter(
            "WerB", [128, n_used * HC, D_OUT], bf16, isOutput=False
        )
    b1r = nc.declare_dram_parameter("b1r", [128, KC], f32, isOutput=False)
    ber = nc.declare_dram_parameter("ber", [128, N_EXP], f32, isOutput=False)
    outT = nc.declare_dram_parameter("outT", [128, R], bf16, isOutput=True)

    nbundle = len(bundles)

    with tile.TileContext(nc) as tc:
        with (
            tc.tile_pool(name="wpool", bufs=1) as wpool,
            tc.tile_pool(name="xpool", bufs=PREFETCH + 2) as xpool,
            tc.tile_pool(name="hpool", bufs=6) as hpool,
            tc.tile_pool(name="opool", bufs=4) as opool,
            tc.tile_pool(name="ph", bufs=3, space=bass.MemorySpace.PSUM) as php,
            tc.tile_pool(name="po", bufs=2, space=bass.MemorySpace.PSUM) as pop,
        ):
            # ---- PE warm-up: dummy matmuls on a zeroed SBUF tile keep
            # the HAM activity monitor busy so the first real matmuls
            # run at 2.4 GHz instead of the cold 1.2 GHz (saved ~4us of
            # throttle in the baseline trace).  They write a po-tag PSUM
            # slot that is long free before the first real expert head.
            wrm = wpool.tile([128, 512], f8, tag="wrm")
            nc.vector.memset(wrm[:], 0.0)
            pwm = pop.tile([128, 256], f32, tag="po", name="pwm")
            for _ in range(N_WARM):
                nc.tensor.matmul(
                    pwm[:], wrm[:, 0:128], wrm[:, 256:512], start=True, stop=True
                )
            # dummy activation pulls the ~1.3us ACT_TABLE_LOAD (sigmoid
            # table set, which also covers Relu) into the idle warm-up
            # window instead of blocking the first real scalar ReLU
            act_warm = wpool.tile([128, 1], bf16, tag="actw")
            nc.scalar.activation(
                act_warm[:],
                wrm[:, 0:1],
                mybir.ActivationFunctionType.Sigmoid,
            )

            # ---- first loads.  Early DMA bandwidth is ramp-limited and
            # shared, so the critical first matmul inputs must not race
            # the prefetch pantry: W1 rides the sync queue ALONE (outs
            # join it much later), while biases, every x tile, and the
            # expert weights ride gpsimd strictly in need order — queue
            # FIFO serializes the pantry BEHIND x0 instead of beside
            # it.  The PE warm-up bridges until W1/x0 land, so compute
            # starts warm with the pantry filling behind it.
            w1_sb = wpool.tile([128, KC, D_H], f8, tag="w1")
            nc.sync.dma_start(w1_sb[:], xT[:, :, 0:D_H])

            x_tiles = {}

            def x_dma(bi, eng):
                bundle = bundles[bi]
                blen = sum(g[1] for g in bundle)
                g0 = bundle[0][0]
                t = xpool.tile([128, KC, blen], f8, tag="x", name=f"x{bi}")
                eng.dma_start(t[:], xT[:, :, D_H + g0 : D_H + g0 + blen])
                x_tiles[bi] = t

            def x_eng(bi):
                return nc.gpsimd

            b1_sb = wpool.tile([128, KC], f32, tag="b1")
            be_sb = wpool.tile([128, N_EXP], f32, tag="be")
            nc.gpsimd.dma_start(b1_sb[:], b1r[:])
            nc.gpsimd.dma_start(be_sb[:], ber[:])
            x_dma(0, nc.gpsimd)

            we_cuts = sorted(set(c for c in (0, 1, 3, 7, n_used) if c <= n_used))
            wpairs = HC if EXPERT_MODE == "fp8" else 2
            weA_chunks = []  # (lo, hi, tile) fp8
            weB_chunks = []  # (lo, hi, tile) bf16

            def emit_we(ci):
                lo, hi = we_cuts[ci], we_cuts[ci + 1]
                if EXPERT_MODE in ("fp8", "mixed"):
                    wa = wpool.tile(
                        [128, (hi - lo) * wpairs, D_OUT], f8, tag=f"weA{lo}"
                    )
                    nc.gpsimd.dma_start(wa[:], WerA[:, lo * wpairs : hi * wpairs, :])
                    weA_chunks.append((lo, hi, wa))
                if EXPERT_MODE in ("mixed", "bf16"):
                    nB = 2 if EXPERT_MODE == "mixed" else HC
                    wb = wpool.tile([128, (hi - lo) * nB, D_OUT], bf16, tag=f"weB{lo}")
                    nc.gpsimd.dma_start(wb[:], WerB[:, lo * nB : hi * nB, :])
                    weB_chunks.append((lo, hi, wb))

            n_chunks = len(we_cuts) - 1
            # need-order interleave; x1 rides sync behind W1 so the
            # early x feed uses both queues' ramp bandwidth
            if nbundle > 1:
                x_dma(1, nc.sync)
            if n_chunks > 0:
                emit_we(0)
            if nbundle > 2:
                x_dma(2, nc.gpsimd)
            if n_chunks > 1:
                emit_we(1)
            if nbundle > 3:
                x_dma(3, nc.gpsimd)
            for ci in range(2, n_chunks):
                emit_we(ci)

            def we_apA(e, j):
                """fp8 DoubleRow pair j (contraction chunks 2j, 2j+1)."""
                s = slot_of[e]
                for lo, hi, w in weA_chunks:
                    if lo <= s < hi:
                        base = (s - lo) * wpairs + 2 * j
                        return w[:, base : base + 2, :]
                raise AssertionError(f"slot {s} not covered")

            def we_apB(e, hc):
                """bf16 single contraction chunk hc."""
                s = slot_of[e]
                nB = 2 if EXPERT_MODE == "mixed" else HC
                off = hc - 2 if EXPERT_MODE == "mixed" else hc
                for lo, hi, w in weB_chunks:
                    if lo <= s < hi:
                        return w[:, (s - lo) * nB + off, :]
                raise AssertionError(f"slot {s} not covered")

            def expert_head(bundle, h2a, h2b):
                """Expert GEMM + sigmoid + out DMA for one bundle.

                Emitted one bundle LATE (software pipelining): the PE
                runs `trunk k+1` while bundle k's last ReLU drains, so
                the expert matmuls never stall on their own bundle's
                PSUM drain.
                """
                for gi, (gstart, glen_, segs) in enumerate(bundle):
                    po = pop.tile([128, glen_], f32, tag="po", name="po")
                    o_bf = opool.tile([128, glen_], bf16, tag="o", name="o")
                    for e, s0, slen in segs:
                        if EXPERT_MODE == "fp8":
                            for j in range(2):
                                nc.tensor.matmul(
                                    po[:, s0 : s0 + slen],
                                    we_apA(e, j),
                                    h2a[:, 2 * j : 2 * j + 2, gi, s0 : s0 + slen],
                                    start=(j == 0),
                                    stop=(j == 1),
                                    perf_mode=DR,
                                )
                        elif EXPERT_MODE == "mixed":
                            nc.tensor.matmul(
                                po[:, s0 : s0 + slen],
                                we_apA(e, 0),
                                h2a[:, 0:2, gi, s0 : s0 + slen],
                                start=True,
                                stop=False,
                                perf_mode=DR,
                            )
                            for hc in (2, 3):
                                nc.tensor.matmul(
                                    po[:, s0 : s0 + slen],
                                    we_apB(e, hc),
                                    h2b[:, hc - 2, gi, s0 : s0 + slen],
                                    start=False,
                                    stop=(hc == 3),
                                )
                        else:
                            for hc in range(HC):
                                nc.tensor.matmul(
                                    po[:, s0 : s0 + slen],
                                    we_apB(e, hc),
                                    h2b[:, hc, gi, s0 : s0 + slen],
                                    start=(hc == 0),
                                    stop=(hc == HC - 1),
                                )
                    for e, s0, slen in segs:
                        nc.scalar.activation(
                            o_bf[:, s0 : s0 + slen],
                            po[:, s0 : s0 + slen],
                            mybir.ActivationFunctionType.Sigmoid,
                            bias=be_sb[:, e : e + 1],
                            scale=1.0 / WE_SCALE,
                        )
                    nc.sync.dma_start(outT[:, gstart : gstart + glen_], o_bf[:])

            pending = None  # (bundle, h2a, h2b) awaiting its expert head
            for bi, bundle in enumerate(bundles):
                nb = len(bundle)
                glen = bundle[0][1]
                xt = x_tiles.pop(bi)
                offs = [0]
                for _, glen_, _ in bundle:
                    offs.append(offs[-1] + glen_)

                if EXPERT_MODE == "fp8":
                    h2a = hpool.tile([128, HC, nb, glen], f8, tag="h", name="h2a")
                    h2b = None
                elif EXPERT_MODE == "mixed":
                    h2a = hpool.tile([128, 2, nb, glen], f8, tag="ha", name="h2a")
                    h2b = hpool.tile([128, 2, nb, glen], bf16, tag="hb", name="h2b")
                else:
                    h2a = None
                    h2b = hpool.tile([128, HC, nb, glen], bf16, tag="h", name="h2b")

                for hc in range(HC):
                    ph = php.tile([128, nb, glen], f32, tag="ph", name="ph")
                    for k2 in range(KC // 2):
                        for gi in range(nb):
                            nc.tensor.matmul(
                                ph[:, gi, :],
                                w1_sb[:, 2 * k2 : 2 * k2 + 2, hc * 128 : (hc + 1) * 128],
                                xt[
                                    :,
                                    2 * k2 : 2 * k2 + 2,
                                    offs[gi] : offs[gi + 1],
                                ],
                                start=(k2 == 0),
                                stop=(k2 == KC // 2 - 1),
                                perf_mode=DR,
                            )
                    # relu(psum + b1): split across the two PSUM-drain
                    # engines.  ScalarE also runs the sigmoids, so it
                    # gets hc 3 only on odd bundles and hc 2-3 on even
                    # ones (~3.1us/bundle each, under the PE's ~4.4).
                    if EXPERT_MODE == "fp8":
                        dst = h2a[:, hc]
                    elif EXPERT_MODE == "mixed":
                        dst = h2a[:, hc] if hc < 2 else h2b[:, hc - 2]
                    else:
                        dst = h2b[:, hc]
                    # single-group bundles have half the PE work but the
                    # same drain volume per row group — keep the split
                    # even there; trailing singles lean on vector since
                    # scalar is still chewing the last pair's sigmoids
                    if nb == 1:
                        n_vec = 3 if bi >= 2 else 2
                    else:
                        n_vec = 3 if bi % 2 else 2
                    if hc < n_vec:
                        nc.vector.tensor_scalar(
                            dst,
                            ph[:],
                            b1_sb[:, hc : hc + 1],
                            0.0,
                            mybir.AluOpType.add,
                            mybir.AluOpType.max,
                        )
                    else:
                        nc.scalar.activation(
                            dst,
                            ph[:],
                            mybir.ActivationFunctionType.Relu,
                            bias=b1_sb[:, hc : hc + 1],
                        )
                    if hc == 0:
                        # just-in-time prefetch keeps each DMA queue's
                        # issue order aligned with need order (an x
                        # issue queued behind a not-yet-ready wait would
                        # block that engine's later out issues).
                        bi2 = bi + PREFETCH
                        if bi2 < nbundle:
                            x_dma(bi2, x_eng(bi2))
                        if pending is not None:
                            # pipelined expert head for the PREVIOUS
                            # bundle, wedged between trunk chunks: its
                            # inputs are long since ready and its
                            # sigmoids precede this bundle's scalar
                            # ReLUs in program order.
                            expert_head(*pending)
                            pending = None

                pending = (bundle, h2a, h2b)
            if pending is not None:
                expert_head(*pending)

    _strip_exit_barriers(nc)
    _split_waits(nc)
    return nc


def kernel(x, num, c, W1, b1, We, be):
    global LAST_RESULTS
    x = np.ascontiguousarray(np.asarray(x, dtype=np.float32))
    W1 = np.asarray(W1, dtype=np.float32)
    b1 = np.asarray(b1, dtype=np.float32)
    We = np.asarray(We, dtype=np.float32)
    be = np.asarray(be, dtype=np.float32)
    num = np.asarray(num).astype(np.int64)
    c = np.asarray(c).astype(np.int64)

    # ---- host routing: sort rows by expert, pad experts to mult of 8 ----
    e = c[num]  # [B]
    order = np.argsort(e, kind="stable")
    e_sorted = e[order]
    counts = np.bincount(e_sorted, minlength=N_EXP)

    perm_parts = []
    local_counts = []  # (expert, m_e) per present expert, in id order
    pos = 0
    for ex in range(N_EXP):
        n = int(counts[ex])
        if n == 0:
            continue
        idx = order[pos : pos + n]
        pos += n
        pad = (-n) % NCORES
        if pad:
            idx = np.concatenate([idx, np.repeat(idx[-1], pad)])
        perm_parts.append(idx)
        local_counts.append((ex, (n + pad) // NCORES))
    perm = np.concatenate(perm_parts)
    R = perm.size // NCORES

    # ---- per-group expert segments (identical on every core) ----
    bounds = []  # (expert, local_start, local_end)
    s = 0
    for ex, m in local_counts:
        bounds.append((ex, s, s + m))
        s += m
    assert s == R

    # all-512 groups plus a small tail group; the tail keeps the
    # pipeline's unhidden end (last expert head + sigmoid + out DMA)
    # short.  If R divides evenly, split the last 512 into 448+64.
    rem = R % GROUP
    glens = [GROUP] * (R // GROUP) + ([rem] if rem else [])
    if not rem and len(glens) > 1:
        glens[-1:] = [448, 64]

    groups = []
    g = 0
    for glen in glens:
        segs = []
        for ex, b0, b1_ in bounds:
            lo = max(b0, g)
            hi = min(b1_, g + glen)
            if lo < hi:
                segs.append((ex, lo - g, hi - lo))
        groups.append((g, glen, segs))
        g += glen
    assert g == R

    # bundles: first two groups single (fast ramp while the DMA queues
    # spin up), then consecutive equal-length 512 pairs, tail single.
    bundles = []
    i = 0
    while i < len(groups):
        if (
            i >= 2
            and i + 1 < len(groups)
            and groups[i][1] == GROUP
            and groups[i + 1][1] == GROUP
        ):
            bundles.append([groups[i], groups[i + 1]])
            i += 2
        else:
            bundles.append([groups[i]])
            i += 1

    # experts in first-use order (must match _build_graph's slot map)
    expert_order = []
    for _, _, segs in groups:
        for ex, _, _ in segs:
            if ex not in expert_order:
                expert_order.append(ex)
    n_used = len(expert_order)

    # ---- host layout prep ----
    W1r = np.ascontiguousarray(
        W1.reshape(KC, 128, D_H).transpose(1, 0, 2)
    ).astype(F8)  # [128, KC, D_H] fp8
    Wer4 = np.ascontiguousarray(
        (We[expert_order] * WE_SCALE)
        .reshape(n_used, HC, 128, D_OUT)
        .transpose(2, 0, 1, 3)
    )  # [128, n_used, HC, D_OUT] f32, scaled by WE_SCALE
    b1r = np.ascontiguousarray(b1.reshape(KC, 128).T)  # [128, KC]
    ber = np.ascontiguousarray(be.T)  # [128, N_EXP]

    weights = {"b1r": b1r, "ber": ber}
    if EXPERT_MODE == "fp8":
        weights["WerA"] = np.ascontiguousarray(
            Wer4.reshape(128, n_used * HC, D_OUT)
        ).astype(F8)
    elif EXPERT_MODE == "mixed":
        weights["WerA"] = np.ascontiguousarray(
            Wer4[:, :, 0:2, :].reshape(128, n_used * 2, D_OUT)
        ).astype(F8)
        weights["WerB"] = np.ascontiguousarray(
            Wer4[:, :, 2:4, :].reshape(128, n_used * 2, D_OUT)
        ).astype(ml_dtypes.bfloat16)
    else:
        weights["WerB"] = np.ascontiguousarray(
            Wer4.reshape(128, n_used * HC, D_OUT)
        ).astype(ml_dtypes.bfloat16)

    # quantize x once, then shuffle bytes per core; each core's xT is
    # prefixed with W1 (same [128, KC, *] fp8 layout) so W1 rides the
    # same tensor as the x stream.
    x8 = x.astype(F8)  # [B, 512]
    in_maps = []
    for i in range(NCORES):
        xi = x8[perm[i::NCORES]]  # [R, 512] fp8
        xTi = xi.T.reshape(KC, 128, R).transpose(1, 0, 2)  # [128, KC, R]
        xw = np.concatenate([W1r, xTi], axis=2)  # [128, KC, D_H + R]
        in_maps.append({"xT": np.ascontiguousarray(xw), **weights})

    # ---- build + run (retry: the device occasionally throws a transient
    # NRT_EXEC_UNIT_UNRECOVERABLE fault; results are lazy jax arrays, so
    # materialize inside the retry to actually catch it) ----
    nc = _build_graph(R, bundles, expert_order)
    outs = None
    for attempt in range(3):
        try:
            res = bass_utils.run_bass_kernel_spmd(
                nc, in_maps, core_ids=list(range(NCORES))
            )
            outs = [
                np.asarray(res.results[i]["outT"]) for i in range(NCORES)
            ]
            break
        except Exception:
            if attempt == 2:
                raise
    LAST_RESULTS = res

    # ---- unshard: scatter rows back (pad rows are dups -> idempotent) ----
    out = np.empty((B, D_OUT), dtype=np.float32)
    for i in range(NCORES):
        out[perm[i::NCORES]] = outs[i].T.astype(np.float32)
    return out


# revision 42
# speedup vs baseline: 1.0087x; 1.0087x over previous
"""MoE routing kernel for 8 TRN2 NeuronCores.

reference:
    h = relu(x @ W1 + b1)            # [B, 512]
    e = c[num]                       # [B] expert ids
    out = sigmoid(h @ We[e] + be[e]) # [B, 128]

Strategy: data-parallel over B with host-side expert sort.  Rows are
stable-sorted by expert id, each expert's row count is padded to a
multiple of 8, and the sorted rows are dealt round-robin to the 8 cores.
Because every expert boundary lands on a multiple of 8 globally, all 8
cores see the *same* local expert-boundary structure, so one SPMD graph
(with per-512-row-group expert segments baked in as compile-time
constants) is valid for every core.  x is pre-transposed on the host so
the device contracts over the partition axis with zero on-device
transposes; the device returns out^T in bf16 which the host transposes
back to f32.

Both GEMMs run in fp8 e4m3 with DoubleRow perf mode (2 contraction rows
per cycle): x/W1 quantized host-side, h quantized on the fly by the
PSUM-drain engines, We quantized host-side with a x16 scale (folded
back out via the sigmoid's scale operand) to dodge the e4m3 subnormal
floor.  The ReLU+bias drain is split across VectorE (hc 0-1) and
ScalarE (hc 2-3) so neither engine gates the fp8-rate PE; sigmoid+bias
runs on ScalarE.  ~48 dummy matmuls on zeroed SBUF warm the PE's HAM
clock gate during the initial DMA wait, and the first loads ride three
parallel HWDGE queues (sync/gpsimd/scalar) so queue spin-up and the
W1/x0 transfers overlap.
"""

import ml_dtypes
import numpy as np

import concourse.bass as bass
import concourse.mybir as mybir
from concourse import tile
from concourse import bass_utils

B, D_IN, D_H, D_OUT, N_EXP = 65536, 512, 512, 128, 16
NCORES = 8
GROUP = 512  # rows per matmul group (one PSUM bank of fp32)
KC = D_IN // 128   # 4 contraction chunks for the trunk
HC = D_H // 128    # 4 contraction chunks for the expert matmul

F8 = ml_dtypes.float8_e4m3  # TRN fp8_exp4 (max +-240), bit-compatible

# "fp8": expert GEMM fully fp8 DoubleRow (fastest, rel err ~1.97e-2)
# "mixed": hc 0-1 fp8 DoubleRow + hc 2-3 bf16 (rel err ~1.7e-2)
# "bf16": expert GEMM fully bf16 (rel err ~1.4e-2)
EXPERT_MODE = "fp8"
WE_SCALE = 16.0  # host-side We scale; 1/WE_SCALE folded into sigmoid
# Dummy matmuls bridging the PE from preamble-end (~8.4us) to the first
# x/W1 data (~12us): keeps the HAM activity monitor busy so the real
# matmuls start at 2.4 GHz instead of cold 1.2 GHz, at zero cost while
# the DMA queues ramp.  ~213ns each (N=256 cold).
N_WARM = 23
PREFETCH = 4     # x tile for bundle k+PREFETCH issued during bundle k

# test.py introspection: the last BassKernelResults (for exec_time_ns)
LAST_RESULTS = None

# If profiling is enabled via BASS_TRACE, keep artifacts local (the default
# upload path needs a remote bucket this environment may not have).
bass_utils.upload_artifacts = lambda tmpdir: tmpdir


def _split_waits(nc, limit=1):
    """Walrus's CoreV3 CTRL codegen rejects instructions carrying more
    than one sem wait; spread extras onto preceding same-engine NoOps."""
    for f in nc.m.functions:
        for bb in f.blocks:
            insts = list(bb.instructions)
            out = []
            changed = False
            for ins in insts:
                si = ins.sync_info
                waits = list(si.on_wait) if si and si.on_wait else []
                if len(waits) > limit:
                    extra, keep = waits[:-limit], waits[-limit:]
                    for i in range(0, len(extra), limit):
                        out.append(
                            mybir.InstNoOp(
                                name=f"{ins.name}-ws{i}",
                                engine=ins.engine,
                                ins=[],
                                outs=[],
                                sync_info=mybir.SyncInfo(
                                    on_wait=extra[i : i + limit], on_update=[]
                                ),
                            )
                        )
                    ins.sync_info = mybir.SyncInfo(
                        on_wait=keep,
                        on_update=list(si.on_update) if si.on_update else [],
                    )
                    changed = True
                out.append(ins)
            if changed:
                bb.instructions[:] = out


def _strip_exit_barriers(nc):
    """Drop Tile's exit-time double all-engine barrier + DMA-queue reset
    + semaphore clear (~8us of measured postamble, inside the profiled
    exec window).  The NEFF executes once per kernel() call, so
    inter-execution semaphore hygiene is dead weight.  The final sync
    drain — which waits out the global tile clock, including the
    out-DMA completion sems — is kept, so outputs are still complete
    when the program ends."""
    for f in nc.m.functions:
        for bb in f.blocks:
            if not bb.name.endswith("_end"):
                continue
            insts = list(bb.instructions)
            cut = None
            for idx, ins in enumerate(insts):
                if (
                    isinstance(ins, mybir.InstDrain)
                    and ins.engine == mybir.EngineType.SP
                ):
                    cut = idx
                    break
            if cut is not None:
                bb.instructions[:] = insts[: cut + 1]


def _build_graph(R, bundles, expert_order):
    """Build the per-core Bass graph.

    R: local rows per core (multiple of 8).
    bundles: list of bundles; a bundle is a list of (gstart, glen, segs)
    with segs = [(expert, s0, slen)] local to the group.
    expert_order: used experts in first-use order; We is packed in this
    order host-side so the early slots can land first.
    """
    f32 = mybir.dt.float32
    bf16 = mybir.dt.bfloat16
    f8 = mybir.dt.float8e4
    DR = mybir.MatmulPerfMode.DoubleRow

    n_used = len(expert_order)
    slot_of = {e: s for s, e in enumerate(expert_order)}

    nc = bass.Bass()
    # xT carries W1 as a same-layout prefix: W1 = xT[:, :, :D_H].
    xT = nc.declare_dram_parameter("xT", [128, KC, D_H + R], f8, isOutput=False)
    if EXPERT_MODE == "fp8":
        WerA = nc.declare_dram_parameter(
            "WerA", [128, n_used * HC, D_OUT], f8, isOutput=False
        )
        WerB = None
    elif EXPERT_MODE == "mixed":
        WerA = nc.declare_dram_parameter(
            "WerA", [128, n_used * 2, D_OUT], f8, isOutput=False
        )
        WerB = nc.declare_dram_parameter(
            "WerB", [128, n_used * 2, D_OUT], bf16, isOutput=False
        )
    else:
        WerA = None
        WerB = nc.declare_dram_parameter(
            "WerB", [128, n_used * HC, D_OUT], bf16, isOutput=False
        )
    b1r = nc.declare_dram_parameter("b1r", [128, KC], f32, isOutput=False)
    ber = nc.declare_dram_parameter("ber", [128, N_EXP], f32, isOutput=False)
    outT = nc.declare_dram_parameter("outT", [128, R], bf16, isOutput=True)

    nbundle = len(bundles)

    with tile.TileContext(nc) as tc:
        with (
            tc.tile_pool(name="wpool", bufs=1) as wpool,
            tc.tile_pool(name="xpool", bufs=PREFETCH + 1) as xpool,
            tc.tile_pool(name="hpool", bufs=4) as hpool,
            tc.tile_pool(name="opool", bufs=4) as opool,
            tc.tile_pool(name="ph", bufs=3, space=bass.MemorySpace.PSUM) as php,
            tc.tile_pool(name="po", bufs=2, space=bass.MemorySpace.PSUM) as pop,
        ):
            # ---- PE warm-up: dummy matmuls on a zeroed SBUF tile keep
            # the HAM activity monitor busy so the first real matmuls
            # run at 2.4 GHz instead of the cold 1.2 GHz (saved ~4us of
            # throttle in the baseline trace).  They write a po-tag PSUM
            # slot that is long free before the first real expert head.
            wrm = wpool.tile([128, 512], f8, tag="wrm")
            nc.vector.memset(wrm[:], 0.0)
            pwm = pop.tile([128, 256], f32, tag="po", name="pwm")
            for _ in range(N_WARM):
                nc.tensor.matmul(
                    pwm[:], wrm[:, 0:128], wrm[:, 256:512], start=True, stop=True
                )
            # dummy activation pulls the ~1.3us ACT_TABLE_LOAD (sigmoid
            # table set, which also covers Relu) into the idle warm-up
            # window instead of blocking the first real scalar ReLU
            act_warm = wpool.tile([128, 1], bf16, tag="actw")
            nc.scalar.activation(
                act_warm[:],
                wrm[:, 0:1],
                mybir.ActivationFunctionType.Sigmoid,
            )

            # ---- first loads.  Early DMA bandwidth is ramp-limited and
            # shared, so the critical first matmul inputs must not race
            # the prefetch pantry: W1 rides the sync queue ALONE (outs
            # join it much later), while biases, every x tile, and the
            # expert weights ride gpsimd strictly in need order — queue
            # FIFO serializes the pantry BEHIND x0 instead of beside
            # it.  The PE warm-up bridges until W1/x0 land, so compute
            # starts warm with the pantry filling behind it.
            w1_sb = wpool.tile([128, KC, D_H], f8, tag="w1")
            nc.sync.dma_start(w1_sb[:], xT[:, :, 0:D_H])

            x_tiles = {}

            def x_dma(bi, eng):
                bundle = bundles[bi]
                blen = sum(g[1] for g in bundle)
                g0 = bundle[0][0]
                t = xpool.tile([128, KC, blen], f8, tag="x", name=f"x{bi}")
                eng.dma_start(t[:], xT[:, :, D_H + g0 : D_H + g0 + blen])
                x_tiles[bi] = t

            def x_eng(bi):
                return nc.gpsimd

            b1_sb = wpool.tile([128, KC], f32, tag="b1")
            be_sb = wpool.tile([128, N_EXP], f32, tag="be")
            nc.gpsimd.dma_start(b1_sb[:], b1r[:])
            nc.gpsimd.dma_start(be_sb[:], ber[:])
            x_dma(0, nc.gpsimd)

            we_cuts = sorted(set(c for c in (0, 1, 3, 7, n_used) if c <= n_used))
            wpairs = HC if EXPERT_MODE == "fp8" else 2
            weA_chunks = []  # (lo, hi, tile) fp8
            weB_chunks = []  # (lo, hi, tile) bf16

            def emit_we(ci):
                lo, hi = we_cuts[ci], we_cuts[ci + 1]
                if EXPERT_MODE in ("fp8", "mixed"):
                    wa = wpool.tile(
                        [128, (hi - lo) * wpairs, D_OUT], f8, tag=f"weA{lo}"
                    )
                    nc.gpsimd.dma_start(wa[:], WerA[:, lo * wpairs : hi * wpairs, :])
                    weA_chunks.append((lo, hi, wa))
                if EXPERT_MODE in ("mixed", "bf16"):
                    nB = 2 if EXPERT_MODE == "mixed" else HC
                    wb = wpool.tile([128, (hi - lo) * nB, D_OUT], bf16, tag=f"weB{lo}")
                    nc.gpsimd.dma_start(wb[:], WerB[:, lo * nB : hi * nB, :])
                    weB_chunks.append((lo, hi, wb))

            n_chunks = len(we_cuts) - 1
            # need-order interleave; x1 rides sync behind W1 so the
            # early x feed uses both queues' ramp bandwidth
            if nbundle > 1:
                x_dma(1, nc.sync)
            if n_chunks > 0:
                emit_we(0)
            if nbundle > 2:
                x_dma(2, nc.gpsimd)
            if n_chunks > 1:
                emit_we(1)
            if nbundle > 3:
                x_dma(3, nc.gpsimd)
            for ci in range(2, n_chunks):
                emit_we(ci)

            def we_apA(e, j):
                """fp8 DoubleRow pair j (contraction chunks 2j, 2j+1)."""
                s = slot_of[e]
                for lo, hi, w in weA_chunks:
                    if lo <= s < hi:
                        base = (s - lo) * wpairs + 2 * j
                        return w[:, base : base + 2, :]
                raise AssertionError(f"slot {s} not covered")

            def we_apB(e, hc):
                """bf16 single contraction chunk hc."""
                s = slot_of[e]
                nB = 2 if EXPERT_MODE == "mixed" else HC
                off = hc - 2 if EXPERT_MODE == "mixed" else hc
                for lo, hi, w in weB_chunks:
                    if lo <= s < hi:
                        return w[:, (s - lo) * nB + off, :]
                raise AssertionError(f"slot {s} not covered")

            def expert_head(bundle, h2a, h2b):
                """Expert GEMM + sigmoid + out DMA for one bundle.

                Emitted one bundle LATE (software pipelining): the PE
                runs `trunk k+1` while bundle k's last ReLU drains, so
                the expert matmuls never stall on their own bundle's
                PSUM drain.
                """
                for gi, (gstart, glen_, segs) in enumerate(bundle):
                    po = pop.tile([128, glen_], f32, tag="po", name="po")
                    o_bf = opool.tile([128, glen_], bf16, tag="o", name="o")
                    for e, s0, slen in segs:
                        if EXPERT_MODE == "fp8":
                            for j in range(2):
                                nc.tensor.matmul(
                                    po[:, s0 : s0 + slen],
                                    we_apA(e, j),
                                    h2a[:, 2 * j : 2 * j + 2, gi, s0 : s0 + slen],
                                    start=(j == 0),
                                    stop=(j == 1),
                                    perf_mode=DR,
                                )
                        elif EXPERT_MODE == "mixed":
                            nc.tensor.matmul(
                                po[:, s0 : s0 + slen],
                                we_apA(e, 0),
                                h2a[:, 0:2, gi, s0 : s0 + slen],
                                start=True,
                                stop=False,
                                perf_mode=DR,
                            )
                            for hc in (2, 3):
                                nc.tensor.matmul(
                                    po[:, s0 : s0 + slen],
                                    we_apB(e, hc),
                                    h2b[:, hc - 2, gi, s0 : s0 + slen],
                                    start=False,
                                    stop=(hc == 3),
                                )
                        else:
                            for hc in range(HC):
                                nc.tensor.matmul(
                                    po[:, s0 : s0 + slen],
                                    we_apB(e, hc),
                                    h2b[:, hc, gi, s0 : s0 + slen],
                                    start=(hc == 0),
                                    stop=(hc == HC - 1),
                                )
                    for e, s0, slen in segs:
                        nc.scalar.activation(
                            o_bf[:, s0 : s0 + slen],
                            po[:, s0 : s0 + slen],
                            mybir.ActivationFunctionType.Sigmoid,
                            bias=be_sb[:, e : e + 1],
                            scale=1.0 / WE_SCALE,
                        )
                    nc.sync.dma_start(outT[:, gstart : gstart + glen_], o_bf[:])

            pending = None  # (bundle, h2a, h2b) awaiting its expert head
            for bi, bundle in enumerate(bundles):
                nb = len(bundle)
                glen = bundle[0][1]
                xt = x_tiles.pop(bi)
                offs = [0]
                for _, glen_, _ in bundle:
                    offs.append(offs[-1] + glen_)

                if EXPERT_MODE == "fp8":
                    h2a = hpool.tile([128, HC, nb, glen], f8, tag="h", name="h2a")
                    h2b = None
                elif EXPERT_MODE == "mixed":
                    h2a = hpool.tile([128, 2, nb, glen], f8, tag="ha", name="h2a")
                    h2b = hpool.tile([128, 2, nb, glen], bf16, tag="hb", name="h2b")
                else:
                    h2a = None
                    h2b = hpool.tile([128, HC, nb, glen], bf16, tag="h", name="h2b")

                for hc in range(HC):
                    ph = php.tile([128, nb, glen], f32, tag="ph", name="ph")
                    for k2 in range(KC // 2):
                        for gi in range(nb):
                            nc.tensor.matmul(
                                ph[:, gi, :],
                                w1_sb[:, 2 * k2 : 2 * k2 + 2, hc * 128 : (hc + 1) * 128],
                                xt[
                                    :,
                                    2 * k2 : 2 * k2 + 2,
                                    offs[gi] : offs[gi + 1],
                                ],
                                start=(k2 == 0),
                                stop=(k2 == KC // 2 - 1),
                                perf_mode=DR,
                            )
                    # relu(psum + b1): split across the two PSUM-drain
                    # engines.  ScalarE also runs the sigmoids, so it
                    # gets hc 3 only on odd bundles and hc 2-3 on even
                    # ones (~3.1us/bundle each, under the PE's ~4.4).
                    if EXPERT_MODE == "fp8":
                        dst = h2a[:, hc]
                    elif EXPERT_MODE == "mixed":
                        dst = h2a[:, hc] if hc < 2 else h2b[:, hc - 2]
                    else:
                        dst = h2b[:, hc]
                    # single-group bundles have half the PE work but the
                    # same drain volume per row group — keep the split
                    # even there; trailing singles lean on vector since
                    # scalar is still chewing the last pair's sigmoids
                    if nb == 1:
                        n_vec = 3 if bi >= 2 else 2
                    else:
                        n_vec = 3 if bi % 2 else 2
                    if hc < n_vec:
                        nc.vector.tensor_scalar(
                            dst,
                            ph[:],
                            b1_sb[:, hc : hc + 1],
                            0.0,
                            mybir.AluOpType.add,
                            mybir.AluOpType.max,
                        )
                    else:
                        nc.scalar.activation(
                            dst,
                            ph[:],
                            mybir.ActivationFunctionType.Relu,
                            bias=b1_sb[:, hc : hc + 1],
                        )
                    if hc == 0:
                        # just-in-time prefetch keeps each DMA queue's
                        # issue order aligned with need order (an x
                        # issue queued behind a not-yet-ready wait would
                        # block that engine's later out issues).
                        bi2 = bi + PREFETCH
                        if bi2 < nbundle:
                            x_dma(bi2, x_eng(bi2))
                        if pending is not None:
                            # pipelined expert head for the PREVIOUS
                            # bundle, wedged between trunk chunks: its
                            # inputs are long since ready and its
                            # sigmoids precede this bundle's scalar
                            # ReLUs in program order.
                            expert_head(*pending)
                            pending = None

                pending = (bundle, h2a, h2b)
            if pending is not None:
                expert_head(*pending)

    _strip_exit_barriers(nc)
    _split_waits(nc)
    return nc


def kernel(x, num, c, W1, b1, We, be):
    global LAST_RESULTS
    x = np.ascontiguousarray(np.asarray(x, dtype=np.float32))
    W1 = np.asarray(W1, dtype=np.float32)
    b1 = np.asarray(b1, dtype=np.float32)
    We = np.asarray(We, dtype=np.float32)
    be = np.asarray(be, dtype=np.float32)
    num = np.asarray(num).astype(np.int64)
    c = np.asarray(c).astype(np.int64)

    # ---- host routing: sort rows by expert, pad experts to mult of 8 ----
    e = c[num]  # [B]
    order = np.argsort(e, kind="stable")
    e_sorted = e[order]
    counts = np.bincount(e_sorted, minlength=N_EXP)

    perm_parts = []
    local_counts = []  # (expert, m_e) per present expert, in id order
    pos = 0
    for ex in range(N_EXP):
        n = int(counts[ex])
        if n == 0:
            continue
        idx = order[pos : pos + n]
        pos += n
        pad = (-n) % NCORES
        if pad:
            idx = np.concatenate([idx, np.repeat(idx[-1], pad)])
        perm_parts.append(idx)
        local_counts.append((ex, (n + pad) // NCORES))
    perm = np.concatenate(perm_parts)
    R = perm.size // NCORES

    # ---- per-group expert segments (identical on every core) ----
    bounds = []  # (expert, local_start, local_end)
    s = 0
    for ex, m in local_counts:
        bounds.append((ex, s, s + m))
        s += m
    assert s == R

    # all-512 groups plus a small tail group; the tail keeps the
    # pipeline's unhidden end (last expert head + sigmoid + out DMA)
    # short.  If R divides evenly, split the last 512 into 448+64.
    rem = R % GROUP
    glens = [GROUP] * (R // GROUP) + ([rem] if rem else [])
    if not rem and len(glens) > 1:
        glens[-1:] = [448, 64]

    groups = []
    g = 0
    for glen in glens:
        segs = []
        for ex, b0, b1_ in bounds:
            lo = max(b0, g)
            hi = min(b1_, g + glen)
            if lo < hi:
                segs.append((ex, lo - g, hi - lo))
        groups.append((g, glen, segs))
        g += glen
    assert g == R

    # bundles: first two groups single (fast ramp while the DMA queues
    # spin up), then consecutive equal-length 512 pairs, tail single.
    bundles = []
    i = 0
    while i < len(groups):
        if (
            i >= 2
            and i + 1 < len(groups)
            and groups[i][1] == GROUP
            and groups[i + 1][1] == GROUP
        ):
            bundles.append([groups[i], groups[i + 1]])
            i += 2
        else:
            bundles.append([groups[i]])
            i += 1

    # experts in first-use order (must match _build_graph's slot map)
    expert_order = []
    for _, _, segs in groups:
        for ex, _, _ in segs:
            if ex not in expert_order:
                expert_order.append(ex)
    n_used = len(expert_order)

    # ---- host layout prep ----
    W1r = np.ascontiguousarray(
        W1.reshape(KC, 128, D_H).transpose(1, 0, 2)
    ).astype(F8)  # [128, KC, D_H] fp8
    Wer4 = np.ascontiguousarray(
        (We[expert_order] * WE_SCALE)
        .reshape(n_used, HC, 128, D_OUT)
        .transpose(2, 0, 1, 3)
    )  # [128, n_used, HC, D_OUT] f32, scaled by WE_SCALE
    b1r = np.ascontiguousarray(b1.reshape(KC, 128).T)  # [128, KC]
    ber = np.ascontiguousarray(be.T)  # [128, N_EXP]

    weights = {"b1r": b1r, "ber": ber}
    if EXPERT_MODE == "fp8":
        weights["WerA"] = np.ascontiguousarray(
            Wer4.reshape(128, n_used * HC, D_OUT)
        ).astype(F8)
    elif EXPERT_MODE == "mixed":
        weights["WerA"] = np.ascontiguousarray(
            Wer4[:, :, 0:2, :].reshape(128, n_used * 2, D_OUT)
        ).astype(F8)
        weights["WerB"] = np.ascontiguousarray(
            Wer4[:, :, 2:4, :].reshape(128, n_used * 2, D_OUT)
        ).astype(ml_dtypes.bfloat16)
    else:
        weights["WerB"] = np.ascontiguousarray(
            Wer4.reshape(128, n_used * HC, D_OUT)
        ).astype(ml_dtypes.bfloat16)

    # quantize x once, then shuffle bytes per core; each core's xT is
    # prefixed with W1 (same [128, KC, *] fp8 layout) so W1 rides the
    # same tensor as the x stream.
    x8 = x.astype(F8)  # [B, 512]
    in_maps = []
    for i in range(NCORES):
        xi = x8[perm[i::NCORES]]  # [R, 512] fp8
        xTi = xi.T.reshape(KC, 128, R).transpose(1, 0, 2)  # [128, KC, R]
        xw = np.concatenate([W1r, xTi], axis=2)  # [128, KC, D_H + R]
        in_maps.append({"xT": np.ascontiguousarray(xw), **weights})

    # ---- build + run (retry: the device occasionally throws a transient
    # NRT_EXEC_UNIT_UNRECOVERABLE fault; results are lazy jax arrays, so
    # materialize inside the retry to actually catch it) ----
    nc = _build_graph(R, bundles, expert_order)
    outs = None
    for attempt in range(3):
        try:
            res = bass_utils.run_bass_kernel_spmd(
                nc, in_maps, core_ids=list(range(NCORES))
            )
            outs = [
                np.asarray(res.results[i]["outT"]) for i in range(NCORES)
            ]
            break
        except Exception:
            if attempt == 2:
                raise
    LAST_RESULTS = res

    # ---- unshard: scatter rows back (pad rows are dups -> idempotent) ----
    out = np.empty((B, D_OUT), dtype=np.float32)
    for i in range(NCORES):
        out[perm[i::NCORES]] = outs[i].T.astype(np.float32)
    return out
